# revision 12
# baseline (speedup 1.0000x reference)
"""Trainium2 Bass kernel for batched 64-point DCT (flattened-patch GEMM).

Reference computation: out = x.reshape(b, -1, 64) @ K, reshaped back.
Pure data parallel over 8 NeuronCores: core i handles batch i as a
[49152, 64] x [64, 64] GEMM.

The problem is HBM-bound (per-core: read 12 MB + write 12 MB fp32 at
~358 GB/s/core).  The correctness bar (rel err < 2e-2) leaves orders of
magnitude of slack, so all HBM traffic is carried in bf16: the host
pre-converts the input (and the DCT basis) to bf16, the device computes
bf16 matmuls with fp32 PSUM accumulation, downcasts to bf16 on the
PSUM->SBUF copy, and stores bf16; the host upcasts to fp32.  This
halves the DMA-streaming window vs fp32.

Device layout: both input and output live as [128, n_pairs] matrices
whose partition dim is (patch-parity z, coefficient):

    xdev[(z*64+s), p] = x[2p+z, s]      ydev[(z*64+f), p] = y[2p+z, f]

With that layout the kernel is a single stationary matmul streamed over
the data: stationary = blockdiag(K, K) [128, 128] (loaded once), moving
= 512-pair slabs of x, PSUM out = the matching slab of y.  No on-chip
transposes; the host packs/unpacks both layouts for free.  Each
[128, 2, 512] PSUM tile (2 banks <- 2 matmuls) is downcast to bf16
SBUF staging by one DVE or ACT copy, then stored contiguously
(4 KB per partition per tile).
"""

import numpy as np
import ml_dtypes

import concourse.mybir as mybir
from concourse import bacc
from concourse.bass_utils import run_bass_kernel_spmd
from concourse.tile import TileContext

P = 128    # SBUF partitions
S = 64     # DCT size
W = 512    # moving-operand width per matmul (one PSUM bank)
MM_PER_TILE = 8
N_CORES = 8
PAIRS_PER_TILE = W * MM_PER_TILE      # 4096 pair-columns per macro-tile
HALF = PAIRS_PER_TILE // 2
PATCHES_PER_TILE = 2 * PAIRS_PER_TILE
KPAD = 256  # k DRAM rows padded to 512 B so its DMA descriptors hit line rate
BF16 = mybir.dt.bfloat16
NP_BF16 = ml_dtypes.bfloat16


def build_kernel(n_patches: int):
    assert n_patches % PATCHES_PER_TILE == 0
    n_tiles = n_patches // PATCHES_PER_TILE
    n_pairs = n_patches // 2
    nc = bacc.Bacc(
        "TRN2",
        target_bir_lowering=False,
        debug=False,
        enable_asserts=False,
        num_devices=N_CORES,
    )
    x = nc.dram_tensor("x", [P, n_pairs], BF16, kind="ExternalInput")
    # host-prepared blockdiag(K, K), rows padded to 512 B
    k = nc.dram_tensor("k", [P, KPAD], BF16, kind="ExternalInput")
    y = nc.dram_tensor("y", [P, n_pairs], BF16, kind="ExternalOutput")

    xv = x.ap().rearrange("r (t n) -> t r n", n=PAIRS_PER_TILE)
    yv = y.ap().rearrange("r (t n) -> t r n", n=HALF)

    with TileContext(nc) as tc:
        with (
            tc.tile_pool(name="consts", bufs=1) as consts,
            tc.tile_pool(name="xin", bufs=6) as x_pool,
            tc.tile_pool(name="outsb", bufs=12) as out_pool,
            tc.tile_pool(name="pout", bufs=4, space="PSUM") as pout_pool,
        ):
            kpad = consts.tile([P, KPAD], BF16)
            kblk = kpad[:, :P]
            # kblk heads the Sync queue (it gates every matmul).
            nc.sync.dma_start(out=kpad[:], in_=k.ap())
            # All input loads are dispatched up-front (x_pool holds every
            # tile), split across both HWDGE rings so neither ring can
            # starve the SDMA engines if the other engine's sequencer
            # stalls.
            x_tiles = []
            for ti in range(n_tiles):
                x_tile = x_pool.tile(
                    [P, PAIRS_PER_TILE], BF16, tag="x_tile",
                    name=f"x_body{ti}",
                )
                eng = nc.sync if ti % 2 == 0 else nc.scalar
                eng.dma_start(out=x_tile[:], in_=xv[ti])
                x_tiles.append(x_tile)

            # Warm the PE HAM clock gate during the DMA head so real
            # matmuls run at 2.4 GHz: a few matmuls on a zeroed tile.
            warm = consts.tile([P, P], BF16)
            nc.scalar.memzero(warm[:])
            warm_po = pout_pool.tile([P, 2, W], mybir.dt.float32, tag="po")
            for _ in range(16):
                nc.tensor.matmul(
                    warm_po[:, 0, :P], lhsT=warm[:], rhs=warm[:],
                    start=True, stop=True,
                )

            for ti in range(n_tiles):
                x_tile = x_tiles[ti]
                for half in range(2):
                    out_sb = out_pool.tile([P, HALF], BF16, tag="out_sb")
                    for h in range(2):
                        po = pout_pool.tile(
                            [P, 2, W], mybir.dt.float32, tag="po"
                        )
                        for j in range(2):
                            m = 4 * half + 2 * h + j
                            nc.tensor.matmul(
                                po[:, j, :],
                                lhsT=kblk,
                                rhs=x_tile[:, W * m : W * (m + 1)],
                                start=True,
                                stop=True,
                            )
                        dst = out_sb[:, 2 * W * h : 2 * W * (h + 1)]
                        if h % 2 == 0:
                            nc.vector.tensor_copy(dst, po[:])
                        else:
                            nc.scalar.copy(dst, po[:])
                    # stores alternate rings too (opposite phase from the
                    # input loads)
                    seng = nc.scalar if half == 0 else nc.sync
                    seng.dma_start(out=yv[2 * ti + half], in_=out_sb[:])
    nc.compile()
    return nc


def shard_input(x_core: np.ndarray) -> np.ndarray:
    """[n_patches, 64] fp32 -> [128, n_pairs] bf16, row (z*64+s) = x[2p+z, s]."""
    n_pairs = x_core.shape[0] // 2
    xb = x_core.astype(NP_BF16).reshape(n_pairs, 2, S)
    return np.ascontiguousarray(xb.transpose(1, 2, 0).reshape(P, n_pairs))


def unshard_output(y_dev: np.ndarray, n_patches: int) -> np.ndarray:
    """[128, n_pairs] bf16 device layout -> [n_patches, 64] fp32."""
    n_pairs = n_patches // 2
    y3 = y_dev.astype(np.float32).reshape(2, S, n_pairs)
    return y3.transpose(2, 0, 1).reshape(n_patches, S)


def make_in_maps(x_full: np.ndarray, kmat: np.ndarray):
    b = x_full.shape[0]
    n_patches = x_full[0].size // S
    kblk_host = np.zeros((P, KPAD), dtype=NP_BF16)
    kblk_host[:S, :S] = kmat.astype(NP_BF16)
    kblk_host[S:P, S:P] = kmat.astype(NP_BF16)
    return [
        {"x": shard_input(x_full[i].reshape(n_patches, S)), "k": kblk_host}
        for i in range(b)
    ]


def kernel(inputs, kernel):
    x_full = np.ascontiguousarray(np.asarray(inputs, dtype=np.float32))
    kmat = np.ascontiguousarray(np.asarray(kernel, dtype=np.float32))
    b, c, h, w = x_full.shape
    assert b == N_CORES, f"expected batch {N_CORES}, got {b}"
    n_patches = c * h * w // S
    nc = build_kernel(n_patches)
    in_maps = make_in_maps(x_full, kmat)
    res = run_bass_kernel_spmd(nc, in_maps, core_ids=list(range(N_CORES)))
    out = np.stack(
        [
            unshard_output(res.results[i]["y"], n_patches).reshape(c, h, w)
            for i in range(b)
        ],
        axis=0,
    )
    return out


# revision 14
# speedup vs baseline: 1.0390x; 1.0390x over previous
"""Trainium2 Bass kernel for batched 64-point DCT (flattened-patch GEMM).

Reference computation: out = x.reshape(b, -1, 64) @ K, reshaped back.
Pure data parallel over 8 NeuronCores: core i handles batch i as a
[49152, 64] x [64, 64] GEMM.

The problem is HBM-bound (per-core: read 12 MB + write 12 MB fp32 at
~358 GB/s/core).  The correctness bar (rel err < 2e-2) leaves orders of
magnitude of slack, so all HBM traffic is carried in bf16: the host
pre-converts the input (and the DCT basis) to bf16, the device computes
bf16 matmuls with fp32 PSUM accumulation, downcasts to bf16 on the
PSUM->SBUF copy, and stores bf16; the host upcasts to fp32.  This
halves the DMA-streaming window vs fp32.

Device layout: both input and output live as [128, n_pairs] matrices
whose partition dim is (patch-parity z, coefficient):

    xdev[(z*64+s), p] = x[2p+z, s]      ydev[(z*64+f), p] = y[2p+z, f]

With that layout the kernel is a single stationary matmul streamed over
the data: stationary = blockdiag(K, K) [128, 128] (loaded once), moving
= 512-pair slabs of x, PSUM out = the matching slab of y.  No on-chip
transposes; the host packs/unpacks both layouts for free.  Each
[128, 2, 512] PSUM tile (2 banks <- 2 matmuls) is downcast to bf16
SBUF staging by one DVE or ACT copy, then stored contiguously
(4 KB per partition per tile).
"""

import numpy as np
import ml_dtypes

import concourse.mybir as mybir
from concourse import bacc
from concourse.bass_utils import run_bass_kernel_spmd
from concourse.tile import TileContext

P = 128    # SBUF partitions
S = 64     # DCT size
W = 512    # moving-operand width per matmul (one PSUM bank)
MM_PER_TILE = 8
N_CORES = 8
PAIRS_PER_TILE = W * MM_PER_TILE      # 4096 pair-columns per macro-tile
HALF = PAIRS_PER_TILE // 2
PATCHES_PER_TILE = 2 * PAIRS_PER_TILE
KPAD = 256  # k DRAM rows padded to 512 B so its DMA descriptors hit line rate
BF16 = mybir.dt.bfloat16
NP_BF16 = ml_dtypes.bfloat16


def build_kernel(n_patches: int):
    assert n_patches % PATCHES_PER_TILE == 0
    n_tiles = n_patches // PATCHES_PER_TILE
    n_pairs = n_patches // 2
    nc = bacc.Bacc(
        "TRN2",
        target_bir_lowering=False,
        debug=False,
        enable_asserts=False,
        num_devices=N_CORES,
    )
    x = nc.dram_tensor("x", [P, n_pairs], BF16, kind="ExternalInput")
    # host-prepared blockdiag(K, K), rows padded to 512 B
    k = nc.dram_tensor("k", [P, KPAD], BF16, kind="ExternalInput")
    y = nc.dram_tensor("y", [P, n_pairs], BF16, kind="ExternalOutput")

    xv = x.ap().rearrange("r (t n) -> t r n", n=PAIRS_PER_TILE)
    yv = y.ap().rearrange("r (t n) -> t r n", n=HALF)

    with TileContext(nc) as tc:
        with (
            tc.tile_pool(name="consts", bufs=1) as consts,
            tc.tile_pool(name="xin", bufs=6) as x_pool,
            tc.tile_pool(name="outsb", bufs=12) as out_pool,
            tc.tile_pool(name="pout", bufs=4, space="PSUM") as pout_pool,
        ):
            kpad = consts.tile([P, KPAD], BF16)
            kblk = kpad[:, :P]
            # kblk heads the Sync queue (it gates every matmul).  All six
            # input loads are dispatched up-front right behind it — with
            # bufs=6 none of their dispatches carries a sem wait, so every
            # input descriptor is in the ring within a few us and the SDMA
            # engines can never starve on the input stream.
            nc.sync.dma_start(out=kpad[:], in_=k.ap())
            x_tiles = []
            for ti in range(n_tiles):
                x_tile = x_pool.tile(
                    [P, PAIRS_PER_TILE], BF16, tag="x_tile",
                    name=f"x_body{ti}",
                )
                nc.sync.dma_start(out=x_tile[:], in_=xv[ti])
                x_tiles.append(x_tile)

            for ti in range(n_tiles):
                x_tile = x_tiles[ti]
                for half in range(2):
                    out_sb = out_pool.tile([P, HALF], BF16, tag="out_sb")
                    for h in range(2):
                        po = pout_pool.tile(
                            [P, 2, W], mybir.dt.float32, tag="po"
                        )
                        for j in range(2):
                            m = 4 * half + 2 * h + j
                            nc.tensor.matmul(
                                po[:, j, :],
                                lhsT=kblk,
                                rhs=x_tile[:, W * m : W * (m + 1)],
                                start=True,
                                stop=True,
                            )
                        dst = out_sb[:, 2 * W * h : 2 * W * (h + 1)]
                        if h % 2 == 0:
                            nc.vector.tensor_copy(dst, po[:])
                        else:
                            nc.scalar.copy(dst, po[:])
                    # store on the Scalar hwdge queue so it overlaps the
                    # Sync-queue input stream
                    nc.scalar.dma_start(out=yv[2 * ti + half], in_=out_sb[:])
    nc.compile()
    return nc


def shard_input(x_core: np.ndarray) -> np.ndarray:
    """[n_patches, 64] fp32 -> [128, n_pairs] bf16, row (z*64+s) = x[2p+z, s]."""
    n_pairs = x_core.shape[0] // 2
    xb = x_core.astype(NP_BF16).reshape(n_pairs, 2, S)
    return np.ascontiguousarray(xb.transpose(1, 2, 0).reshape(P, n_pairs))


def unshard_output(y_dev: np.ndarray, n_patches: int) -> np.ndarray:
    """[128, n_pairs] bf16 device layout -> [n_patches, 64] fp32."""
    n_pairs = n_patches // 2
    y3 = y_dev.astype(np.float32).reshape(2, S, n_pairs)
    return y3.transpose(2, 0, 1).reshape(n_patches, S)


def make_in_maps(x_full: np.ndarray, kmat: np.ndarray):
    b = x_full.shape[0]
    n_patches = x_full[0].size // S
    kblk_host = np.zeros((P, KPAD), dtype=NP_BF16)
    kblk_host[:S, :S] = kmat.astype(NP_BF16)
    kblk_host[S:P, S:P] = kmat.astype(NP_BF16)
    return [
        {"x": shard_input(x_full[i].reshape(n_patches, S)), "k": kblk_host}
        for i in range(b)
    ]


def kernel(inputs, kernel):
    x_full = np.ascontiguousarray(np.asarray(inputs, dtype=np.float32))
    kmat = np.ascontiguousarray(np.asarray(kernel, dtype=np.float32))
    b, c, h, w = x_full.shape
    assert b == N_CORES, f"expected batch {N_CORES}, got {b}"
    n_patches = c * h * w // S
    nc = build_kernel(n_patches)
    in_maps = make_in_maps(x_full, kmat)
    res = run_bass_kernel_spmd(nc, in_maps, core_ids=list(range(N_CORES)))
    out = np.stack(
        [
            unshard_output(res.results[i]["y"], n_patches).reshape(c, h, w)
            for i in range(b)
        ],
        axis=0,
    )
    return out
